# revision 12
# baseline (speedup 1.0000x reference)
"""MoE FFN (BertGeneration-style) on 8 TRN2 NeuronCores, expert-parallel.

Problem: 8192 tokens, expert = task_id % 8, per-expert FFN
(768 -> 3072 gelu -> 768) + residual + per-expert LayerNorm.

Strategy: routing (dispatch/combine) is a host-side permutation; each of the
8 cores runs one expert's FFN over its 1024-token block.  Both GEMMs run in
fp8 (e4m3) with MatmulPerfMode.DoubleRow: a [128, 2, 128] stationary tile
contracts K=256 per instruction at 2x the bf16/fp32r MAC rate, and fp8
stationary tiles load 4x faster than fp32r ones (the fp32r baseline was
LdWeights-bound).

Accuracy: weights are pre-scaled by 512 so uniform(+-1/sqrt(H)) values sit in
e4m3's normal range (unscaled, most of W2 would land in subnormals).  The W1
scale is undone by the gelu activation's input scale (1/512); the W2 scale is
ironed out by pre-scaling the residual by 512 -- LayerNorm is scale-invariant
so the output is unchanged.  Measured rel err ~1e-2 vs the fp32 reference.

Schedule notes (from HW traces):
- phase 1 is a single dense matmul stream; its w1 chunk stream is the
  critical DMA path, so w2/xn prefetches are pinned behind phase progress
  (dummy-copy deps) to keep them from oversubscribing HBM mid-phase --
  a starved w1 stream stalls the PE and drops its pstate clock.
- the LayerNorm epilogue is split across engines (residual add: Pool+DVE,
  stats: DVE, sqrt + normalize: Scalar, stores: Sync queue) so the trailing
  chain after the last matmul is short and no single engine backs up.
"""

import sys

if "/opt/trn_rl_repo" not in sys.path:
    sys.path.insert(0, "/opt/trn_rl_repo")

import numpy as np


def _install_axon_hooks_shim():
    """Provide antenv.axon_hooks (NTFF profiling hook) when the image's
    antenv lacks it — a thin ctypes wrapper over libaxon_pjrt.so, matching
    trn_agent_boot.trn_boot._ntff_profile_via_ctypes.  Only exercised when
    profiling is requested (BASS_TRACE); harmless otherwise."""
    import contextlib
    import ctypes
    import types

    try:
        import antenv.axon_hooks  # noqa: F401
        return
    except ImportError:
        pass
    try:
        import antenv
    except ImportError:
        return

    mod = types.ModuleType("antenv.axon_hooks")
    _state = {"hook": None, "init": False}

    def set_axon_ntff_profile_hook(h):
        _state["hook"] = h
        _state["init"] = True

    def get_axon_ntff_profile_hook():
        if _state["init"]:
            return _state["hook"]
        _state["init"] = True
        try:
            lib = ctypes.CDLL("/opt/axon/libaxon_pjrt.so")
        except OSError:
            return None
        if not hasattr(lib, "axon_start_nrt_profile"):
            return None
        lib.axon_start_nrt_profile.argtypes = [
            ctypes.POINTER(ctypes.c_int64), ctypes.c_size_t]
        lib.axon_start_nrt_profile.restype = ctypes.c_int64
        lib.axon_stop_nrt_profile.argtypes = [ctypes.c_char_p]
        lib.axon_stop_nrt_profile.restype = ctypes.c_int64

        @contextlib.contextmanager
        def _hook(output_dir, device_ids):
            import jax
            jax.devices()
            if device_ids:
                ids = (ctypes.c_int64 * len(device_ids))(*device_ids)
                rc = lib.axon_start_nrt_profile(ids, len(device_ids))
            else:
                rc = lib.axon_start_nrt_profile(None, 0)
            if rc != 0:
                raise RuntimeError(f"axon_start_nrt_profile rc={rc}")
            try:
                yield
            finally:
                n = lib.axon_stop_nrt_profile(str(output_dir).encode())
                print(f"profile: {n} file(s) written to {output_dir}")

        _state["hook"] = _hook
        return _hook

    mod.set_axon_ntff_profile_hook = set_axon_ntff_profile_hook
    mod.get_axon_ntff_profile_hook = get_axon_ntff_profile_hook
    sys.modules["antenv.axon_hooks"] = mod
    antenv.axon_hooks = mod


_install_axon_hooks_shim()

E = 8
N = 8192
H = 768
I = 3072
C = N // E        # 1024 tokens per expert/core
MT = C // 128     # 8   m-tiles (token dim per core)
IT = I // 128     # 24  i-tiles (intermediate dim)
T1 = H // 256     # 3   K-tiles of 256 for GEMM1 (hidden dim)
T2 = I // 256     # 12  K-tiles of 256 for GEMM2 (intermediate dim)
EPS = 1e-12
WS = 512.0        # weight pre-scale: keeps uniform(+-1/sqrt(H)) out of
                  # e4m3 subnormals; undone via gelu input scale / LN

_CACHE = {}


def _build_nc(act_name="Gelu"):
    from contextlib import ExitStack

    import concourse.tile as tile
    from concourse import bacc, mybir

    f32 = mybir.dt.float32
    f8 = mybir.dt.float8e4
    AF = mybir.ActivationFunctionType
    act_fn = getattr(AF, act_name)
    ALU = mybir.AluOpType
    DR = mybir.MatmulPerfMode.DoubleRow

    nc = bacc.Bacc("TRN2", target_bir_lowering=False, debug=False, num_devices=8)

    # DoubleRow operand layouts pack the K=256 contraction as (pair j, 128
    # partitions p): element [p, ..., j, ...] holds row k = 256*t + 128*j + p.
    xd = nc.dram_tensor("xd", [128, T1, 2, C], f8, kind="ExternalInput").ap()
    w1 = nc.dram_tensor("w1", [128, IT, T1, 2, 128], f8,
                        kind="ExternalInput").ap()
    w2 = nc.dram_tensor("w2", [128, T2 // 2, 2, 2, H], f8,
                        kind="ExternalInput").ap()
    b1t = nc.dram_tensor("b1t", [128, IT], f32, kind="ExternalInput").ap()
    xn = nc.dram_tensor("xn", [128, MT, H], f32, kind="ExternalInput").ap()
    out = nc.dram_tensor("out", [128, MT, H], f32, kind="ExternalOutput").ap()

    with ExitStack() as ctx:
        tc = ctx.enter_context(tile.TileContext(nc))
        persist = ctx.enter_context(tc.tile_pool(name="persist", bufs=1))
        psum = ctx.enter_context(tc.tile_pool(name="psum", bufs=4, space="PSUM"))
        w1pool = ctx.enter_context(tc.tile_pool(name="w1s", bufs=3))
        zpool = ctx.enter_context(tc.tile_pool(name="zo", bufs=2))
        spool = ctx.enter_context(tc.tile_pool(name="small", bufs=4))

        # hT doubles as GEMM1 output and GEMM2 stationary: [p, t, j, c] is
        # h[i=256t+128j+p, c], exactly DoubleRow's paired-K layout
        hT = persist.tile([128, T2, 2, C], f8, name="hT")
        # x chunk for K-tile 0 lands as two half-token tiles so the first
        # matmuls fire ~0.4us earlier
        xk0h = [persist.tile([128, 2, 512], f8, name=f"xk0{h}", tag=f"xk0{h}")
                for h in range(2)]
        xk = [None] + [persist.tile([128, 2, C], f8, name=f"xk{t}", tag=f"xk{t}")
                       for t in (1, 2)]
        w2s = [persist.tile([128, 2, 2, H], f8, name=f"w2c{t}", tag=f"w2c{t}")
               for t in range(T2 // 2)]
        xns = [persist.tile([128, H], f32, name=f"xn{m}", tag=f"xn{m}")
               for m in range(MT)]
        b1s = persist.tile([128, IT], f32, name="b1s")
        epsT = persist.tile([128, 1], f32, name="epsT")

        nc.vector.memset(epsT, EPS)

        # ---- phase 1: hT = gelu((W1s.T @ xT)/512 + b1) ----
        # startup-critical pair: xk0h[0] leads the gpsimd queue, w1[0] leads
        # the sync queue; nothing else competes for HBM until they land.
        # b1 rides the otherwise-idle scalar queue.
        nc.gpsimd.dma_start(out=xk0h[0], in_=xd[:, 0, :, 0:512])
        nc.gpsimd.dma_start(out=xk0h[1], in_=xd[:, 0, :, 512:C])
        nc.gpsimd.dma_start(out=xk[1], in_=xd[:, 1])
        nc.gpsimd.dma_start(out=xk[2], in_=xd[:, 2])
        nc.scalar.dma_start(out=b1s, in_=b1t)

        for it in range(IT):
            w1t = w1pool.tile([128, T1, 2, 128], f8, name="w1t", tag="w1t")
            nc.sync.dma_start(out=w1t, in_=w1[:, it])
            ph = psum.tile([128, C], f32, name="ph", tag="pt")
            for t in range(T1):
                lhsT = w1t[:, t, :, :]
                for cc in range(4):
                    if t == 0:
                        rhs = xk0h[cc // 2][:, :, (cc % 2) * 256:
                                            (cc % 2) * 256 + 256]
                    else:
                        rhs = xk[t][:, :, cc * 256:(cc + 1) * 256]
                    # PSUM groups are bank-granular (2KB): within a bank
                    # (2 x 256-token chunks) only the first chunk starts the
                    # group and only the last stops it; the middle chunk's
                    # first write lands on pending-zero bytes
                    nc.tensor.matmul(
                        ph[:, cc * 256:(cc + 1) * 256],
                        lhsT=lhsT,
                        rhs=rhs,
                        start=(t == 0 and cc % 2 == 0),
                        stop=(t == T1 - 1 and cc % 2 == 1),
                        perf_mode=DR,
                    )
            nc.scalar.activation(hT[:, it // 2, it % 2, :], ph, act_fn,
                                 bias=b1s[:, it:it + 1], scale=1.0 / WS)
            # W2 chunks stream on the gpsimd queue, pinned behind xk[2]'s
            # arrival (dummy copy -> WAW on the tile) so the scheduler can't
            # hoist them into the startup window where they'd starve the
            # critical first x/w1 chunks.  The pin source must be a tile
            # nothing writes later: reading hT here would give every
            # subsequent gelu a WAR dependency on the copy, backing up the
            # PSUM ring and stalling the PE every 4 i-tiles.
            if it < T2 // 2:
                nc.vector.tensor_copy(
                    out=w2s[it][:, 0, 0, 0:4].bitcast(f32),
                    in_=xk[2][:, 0, 0:4].bitcast(f32))
                nc.gpsimd.dma_start(out=w2s[it], in_=w2[:, it])
            # first two residual blocks prefetch mid-phase-1, pinned behind
            # the last W2 chunks; the rest chain off phase-2 progress below
            if it in (T2 // 2, T2 // 2 + 1):
                mtp = it - T2 // 2
                nc.vector.tensor_copy(
                    out=xns[mtp][:, 0:1],
                    in_=w2s[4 + mtp][:, 0, 0, 0:4].bitcast(f32))
                nc.gpsimd.dma_start(out=xns[mtp], in_=xn[:, mtp])

        # ---- phase 2: y = hT.T @ W2s; z = y + 512*(x + b2); LayerNorm ----
        for mt in range(MT):
            # prefetch the residual block two iterations ahead, pinned
            # behind this iteration's (already-arrived) block
            if mt + 2 < MT:
                nc.vector.tensor_copy(out=xns[mt + 2][:, 0:1],
                                      in_=xns[mt][:, 0:1])
                nc.gpsimd.dma_start(out=xns[mt + 2], in_=xn[:, mt + 2])
            py = psum.tile([128, C], f32, name="py", tag="pt")
            for t in range(T2):
                lhsT = hT[:, t, :, mt * 128:(mt + 1) * 128]
                for hc in range(3):
                    # hc0+hc1 share PSUM bank 0, hc2 owns bank 1
                    nc.tensor.matmul(
                        py[:, hc * 256:(hc + 1) * 256],
                        lhsT=lhsT,
                        rhs=w2s[t // 2][:, t % 2, :,
                                        hc * 256:(hc + 1) * 256],
                        start=(t == 0 and hc != 1),
                        stop=(t == T2 - 1 and hc != 0),
                        perf_mode=DR,
                    )
            # residual add in place (DVE: the Pool engine cannot read PSUM)
            z = xns[mt]
            nc.vector.tensor_add(z, py[:, 0:H], z)
            stats = spool.tile([128, 3, 6], f32, name="stats", tag="stats")
            # g2 first: its half is ready as soon as the DVE add retires
            nc.vector.bn_stats(stats[:, 2], z[:, 512:768])
            nc.vector.bn_stats(stats[:, 0], z[:, 0:256])
            nc.vector.bn_stats(stats[:, 1], z[:, 256:512])
            mv = spool.tile([128, 2], f32, name="mv", tag="mv")
            nc.vector.bn_aggr(mv, stats)
            rstd = spool.tile([128, 1], f32, name="rstd", tag="rstd")
            nc.scalar.activation(rstd, mv[:, 1:2], AF.Sqrt, bias=epsT)
            nc.vector.reciprocal(out=rstd, in_=rstd)
            # normalize as z*rstd + (-mean*rstd) on the Scalar engine,
            # stores on the (idle) sync queue; halves overlap the tail
            nmr = spool.tile([128, 1], f32, name="nmr", tag="nmr")
            nc.vector.tensor_scalar(
                out=nmr, in0=mv[:, 0:1], scalar1=rstd, scalar2=-1.0,
                op0=ALU.mult, op1=ALU.mult)
            zo = zpool.tile([128, H], f32, name="zot", tag="zot")
            for h0 in (0, H // 2):
                sl = slice(h0, h0 + H // 2)
                nc.scalar.activation(zo[:, sl], z[:, sl], AF.Identity,
                                     bias=nmr, scale=rstd)
                nc.sync.dma_start(out=out[:, mt, sl], in_=zo[:, sl])

    nc.compile()
    return nc


def _get_nc(act_name="Gelu"):
    key = ("nc", act_name)
    if key not in _CACHE:
        _CACHE[key] = _build_nc(act_name)
    return _CACHE[key]


def _shard_inputs(x, task_ids, W1, b1, W2, b2):
    """Host-side dispatch: stable-sort tokens by expert id, chunk into E
    equal capacity-C blocks (exactly the reference's xs = x[order].reshape),
    then quantize/pack into the DoubleRow fp8 operand layouts."""
    import ml_dtypes

    f8 = ml_dtypes.float8_e4m3
    expert = (task_ids.astype(np.int64) % E).astype(np.int32)
    order = np.argsort(expert, kind="stable")
    xs = x[order]
    in_maps = []
    for e in range(E):
        xe = xs[e * C:(e + 1) * C]                       # [C, H]
        # xd[p, t, j, c] = fp8(xe[c, 256t + 128j + p])
        xq = np.ascontiguousarray(xe.T).astype(f8)
        xd = xq.reshape(T1, 2, 128, C).transpose(2, 0, 1, 3)
        # w1[p, it, t, j, m] = fp8(512 * W1[256t+128j+p, 128it+m])
        w1q = (W1[e] * WS).astype(f8)
        w1d = w1q.reshape(T1, 2, 128, IT, 128).transpose(2, 3, 0, 1, 4)
        # w2[p, tp, ts, j, h] = fp8(512 * W2[512tp+256ts+128j+p, h])
        w2q = (W2[e] * WS).astype(f8)
        w2d = w2q.reshape(T2 // 2, 2, 2, 128, H).transpose(3, 0, 1, 2, 4)
        b1t = b1[e].reshape(IT, 128).T
        xnv = ((xe + b2[e][None, :]) * WS).reshape(MT, 128, H).transpose(1, 0, 2)
        in_maps.append({
            "xd": np.ascontiguousarray(xd),
            "w1": np.ascontiguousarray(w1d),
            "w2": np.ascontiguousarray(w2d),
            "b1t": np.ascontiguousarray(b1t, dtype=np.float32),
            "xn": np.ascontiguousarray(xnv, dtype=np.float32),
        })
    return in_maps, order


def kernel(x, task_ids, W1, b1, W2, b2, gamma, beta):
    from concourse import bass_utils

    x = np.asarray(x, dtype=np.float32)
    task_ids = np.asarray(task_ids)
    W1 = np.asarray(W1, dtype=np.float32)
    b1 = np.asarray(b1, dtype=np.float32)
    W2 = np.asarray(W2, dtype=np.float32)
    b2 = np.asarray(b2, dtype=np.float32)
    gamma = np.asarray(gamma, dtype=np.float32)
    beta = np.asarray(beta, dtype=np.float32)

    in_maps, order = _shard_inputs(x, task_ids, W1, b1, W2, b2)
    nc = _get_nc()
    res = bass_utils.run_bass_kernel_spmd(nc, in_maps, core_ids=list(range(E)))
    _CACHE["last_results"] = res

    z = np.concatenate(
        [res.results[e]["out"].transpose(1, 0, 2).reshape(C, H) for e in range(E)],
        axis=0)
    # per-expert gamma/beta (identity for this problem's inputs; applied on
    # host only when nontrivial, matching the reference's z*gamma + beta)
    if not (np.all(gamma == 1.0) and np.all(beta == 0.0)):
        blk = np.repeat(np.arange(E), C)  # reference uses capacity blocks
        z = z * gamma[blk] + beta[blk]
    out = np.empty((N, H), dtype=np.float32)
    out[order] = z
    return out


# revision 14
# speedup vs baseline: 1.0819x; 1.0819x over previous
"""MoE FFN (BertGeneration-style) on 8 TRN2 NeuronCores, expert-parallel.

Problem: 8192 tokens, expert = task_id % 8, per-expert FFN
(768 -> 3072 gelu -> 768) + residual + per-expert LayerNorm.

Strategy: routing (dispatch/combine) is a host-side permutation; each of the
8 cores runs one expert's FFN over its 1024-token block.  Both GEMMs run in
fp8 (e4m3) with MatmulPerfMode.DoubleRow: a [128, 2, 128] stationary tile
contracts K=256 per instruction at 2x the bf16/fp32r MAC rate, and fp8
stationary tiles load 4x faster than fp32r ones (the fp32r baseline was
LdWeights-bound).

Accuracy: weights are pre-scaled by 512 so uniform(+-1/sqrt(H)) values sit in
e4m3's normal range (unscaled, most of W2 would land in subnormals).  The W1
scale is undone by the gelu activation's input scale (1/512); the W2 scale is
ironed out by pre-scaling the residual by 512 -- LayerNorm is scale-invariant
so the output is unchanged.  Measured rel err ~1e-2 vs the fp32 reference.

Schedule notes (from HW traces):
- phase 1 is a single dense matmul stream; its w1 chunk stream is the
  critical DMA path, so w2/xn prefetches are pinned behind phase progress
  (dummy-copy deps) to keep them from oversubscribing HBM mid-phase --
  a starved w1 stream stalls the PE and drops its pstate clock.
- the LayerNorm epilogue is split across engines (residual add: Pool+DVE,
  stats: DVE, sqrt + normalize: Scalar, stores: Sync queue) so the trailing
  chain after the last matmul is short and no single engine backs up.
"""

import sys

if "/opt/trn_rl_repo" not in sys.path:
    sys.path.insert(0, "/opt/trn_rl_repo")

import numpy as np


def _install_axon_hooks_shim():
    """Provide antenv.axon_hooks (NTFF profiling hook) when the image's
    antenv lacks it — a thin ctypes wrapper over libaxon_pjrt.so, matching
    trn_agent_boot.trn_boot._ntff_profile_via_ctypes.  Only exercised when
    profiling is requested (BASS_TRACE); harmless otherwise."""
    import contextlib
    import ctypes
    import types

    try:
        import antenv.axon_hooks  # noqa: F401
        return
    except ImportError:
        pass
    try:
        import antenv
    except ImportError:
        return

    mod = types.ModuleType("antenv.axon_hooks")
    _state = {"hook": None, "init": False}

    def set_axon_ntff_profile_hook(h):
        _state["hook"] = h
        _state["init"] = True

    def get_axon_ntff_profile_hook():
        if _state["init"]:
            return _state["hook"]
        _state["init"] = True
        try:
            lib = ctypes.CDLL("/opt/axon/libaxon_pjrt.so")
        except OSError:
            return None
        if not hasattr(lib, "axon_start_nrt_profile"):
            return None
        lib.axon_start_nrt_profile.argtypes = [
            ctypes.POINTER(ctypes.c_int64), ctypes.c_size_t]
        lib.axon_start_nrt_profile.restype = ctypes.c_int64
        lib.axon_stop_nrt_profile.argtypes = [ctypes.c_char_p]
        lib.axon_stop_nrt_profile.restype = ctypes.c_int64

        @contextlib.contextmanager
        def _hook(output_dir, device_ids):
            import jax
            jax.devices()
            if device_ids:
                ids = (ctypes.c_int64 * len(device_ids))(*device_ids)
                rc = lib.axon_start_nrt_profile(ids, len(device_ids))
            else:
                rc = lib.axon_start_nrt_profile(None, 0)
            if rc != 0:
                raise RuntimeError(f"axon_start_nrt_profile rc={rc}")
            try:
                yield
            finally:
                n = lib.axon_stop_nrt_profile(str(output_dir).encode())
                print(f"profile: {n} file(s) written to {output_dir}")

        _state["hook"] = _hook
        return _hook

    mod.set_axon_ntff_profile_hook = set_axon_ntff_profile_hook
    mod.get_axon_ntff_profile_hook = get_axon_ntff_profile_hook
    sys.modules["antenv.axon_hooks"] = mod
    antenv.axon_hooks = mod


_install_axon_hooks_shim()

E = 8
N = 8192
H = 768
I = 3072
C = N // E        # 1024 tokens per expert/core
MT = C // 128     # 8   m-tiles (token dim per core)
IT = I // 128     # 24  i-tiles (intermediate dim)
T1 = H // 256     # 3   K-tiles of 256 for GEMM1 (hidden dim)
T2 = I // 256     # 12  K-tiles of 256 for GEMM2 (intermediate dim)
EPS = 1e-12
WS = 512.0        # weight pre-scale: keeps uniform(+-1/sqrt(H)) out of
                  # e4m3 subnormals; undone via gelu input scale / LN

_CACHE = {}


def _build_nc(act_name="Gelu"):
    from contextlib import ExitStack

    import concourse.tile as tile
    from concourse import bacc, mybir

    f32 = mybir.dt.float32
    f8 = mybir.dt.float8e4
    AF = mybir.ActivationFunctionType
    act_fn = getattr(AF, act_name)
    ALU = mybir.AluOpType
    DR = mybir.MatmulPerfMode.DoubleRow

    nc = bacc.Bacc("TRN2", target_bir_lowering=False, debug=False, num_devices=8)

    # DoubleRow operand layouts pack the K=256 contraction as (pair j, 128
    # partitions p): element [p, ..., j, ...] holds row k = 256*t + 128*j + p.
    xd = nc.dram_tensor("xd", [128, T1, 2, C], f8, kind="ExternalInput").ap()
    w1 = nc.dram_tensor("w1", [128, IT, T1, 2, 128], f8,
                        kind="ExternalInput").ap()
    w2 = nc.dram_tensor("w2", [128, T2 // 2, 2, 2, H], f8,
                        kind="ExternalInput").ap()
    b1t = nc.dram_tensor("b1t", [128, IT], f32, kind="ExternalInput").ap()
    xn = nc.dram_tensor("xn", [128, MT, H], f32, kind="ExternalInput").ap()
    out = nc.dram_tensor("out", [128, MT, H], f32, kind="ExternalOutput").ap()

    with ExitStack() as ctx:
        tc = ctx.enter_context(tile.TileContext(nc))
        persist = ctx.enter_context(tc.tile_pool(name="persist", bufs=1))
        psum = ctx.enter_context(tc.tile_pool(name="psum", bufs=4, space="PSUM"))
        w1pool = ctx.enter_context(tc.tile_pool(name="w1s", bufs=3))
        zpool = ctx.enter_context(tc.tile_pool(name="zo", bufs=3))
        spool = ctx.enter_context(tc.tile_pool(name="small", bufs=8))

        # hT doubles as GEMM1 output and GEMM2 stationary: [p, t, j, c] is
        # h[i=256t+128j+p, c], exactly DoubleRow's paired-K layout
        hT = persist.tile([128, T2, 2, C], f8, name="hT")
        # x chunk for K-tile 0 lands as two half-token tiles so the first
        # matmuls fire ~0.4us earlier
        xk0h = [persist.tile([128, 2, 512], f8, name=f"xk0{h}", tag=f"xk0{h}")
                for h in range(2)]
        xk = [None] + [persist.tile([128, 2, C], f8, name=f"xk{t}", tag=f"xk{t}")
                       for t in (1, 2)]
        w2s = [persist.tile([128, 2, 2, H], f8, name=f"w2c{t}", tag=f"w2c{t}")
               for t in range(T2 // 2)]
        xns = [persist.tile([128, H], f32, name=f"xn{m}", tag=f"xn{m}")
               for m in range(MT)]
        b1s = persist.tile([128, IT], f32, name="b1s")
        epsT = persist.tile([128, 1], f32, name="epsT")

        nc.vector.memset(epsT, EPS)

        # ---- phase 1: hT = gelu((W1s.T @ xT)/512 + b1) ----
        # startup-critical pair: xk0h[0] leads the gpsimd queue, w1[0] leads
        # the sync queue; nothing else competes for HBM until they land.
        # b1 rides the otherwise-idle scalar queue.
        nc.gpsimd.dma_start(out=xk0h[0], in_=xd[:, 0, :, 0:512])
        nc.gpsimd.dma_start(out=xk0h[1], in_=xd[:, 0, :, 512:C])
        nc.gpsimd.dma_start(out=xk[1], in_=xd[:, 1])
        nc.gpsimd.dma_start(out=xk[2], in_=xd[:, 2])
        nc.scalar.dma_start(out=b1s, in_=b1t)

        for it in range(IT):
            w1t = w1pool.tile([128, T1, 2, 128], f8, name="w1t", tag="w1t")
            nc.sync.dma_start(out=w1t, in_=w1[:, it])
            ph = psum.tile([128, C], f32, name="ph", tag="pt")
            for t in range(T1):
                lhsT = w1t[:, t, :, :]
                for cc in range(4):
                    if t == 0:
                        rhs = xk0h[cc // 2][:, :, (cc % 2) * 256:
                                            (cc % 2) * 256 + 256]
                    else:
                        rhs = xk[t][:, :, cc * 256:(cc + 1) * 256]
                    # PSUM groups are bank-granular (2KB): within a bank
                    # (2 x 256-token chunks) only the first chunk starts the
                    # group and only the last stops it; the middle chunk's
                    # first write lands on pending-zero bytes
                    nc.tensor.matmul(
                        ph[:, cc * 256:(cc + 1) * 256],
                        lhsT=lhsT,
                        rhs=rhs,
                        start=(t == 0 and cc % 2 == 0),
                        stop=(t == T1 - 1 and cc % 2 == 1),
                        perf_mode=DR,
                    )
            nc.scalar.activation(hT[:, it // 2, it % 2, :], ph, act_fn,
                                 bias=b1s[:, it:it + 1], scale=1.0 / WS)
            # W2 chunks + first two residual blocks stream on the gpsimd
            # queue as a completion-paced chain: each link's DMA is pinned
            # (dummy copy -> WAW) behind the PREVIOUS link's arrival, first
            # link behind xk[2].  The hardware DMA rings are FIFO in issue
            # order, so an unpaced prefetch burst would head-of-line block
            # the critical w1 stream; pacing keeps at most one 393KB chunk
            # ahead of it.  Pin sources are tiles nothing writes later
            # (reading hT would hand every subsequent gelu a WAR dep,
            # backing up the PSUM ring).
            if it == 0:
                chain = [("w2", i) for i in range(3)] + [("xn", 0)] + \
                        [("w2", i) for i in (3, 4)] + [("xn", 1), ("w2", 5)]
                prev = xk[2][:, 0, 0:4].bitcast(f32)
                for kind, i in chain:
                    if kind == "w2":
                        dst, src = w2s[i], w2[:, i]
                        pin = w2s[i][:, 0, 0, 0:4].bitcast(f32)
                    else:
                        dst, src = xns[i], xn[:, i]
                        pin = xns[i][:, 0:1]
                    nc.vector.tensor_copy(out=pin, in_=prev)
                    nc.gpsimd.dma_start(out=dst, in_=src)
                    prev = pin

        # ---- phase 2: y = hT.T @ W2s; z = y + 512*(x + b2); LayerNorm ----
        for mt in range(MT):
            # prefetch the residual block two iterations ahead, pinned
            # behind this iteration's (already-arrived) block
            if mt + 2 < MT:
                nc.vector.tensor_copy(out=xns[mt + 2][:, 0:1],
                                      in_=xns[mt][:, 0:1])
                nc.gpsimd.dma_start(out=xns[mt + 2], in_=xn[:, mt + 2])
            py = psum.tile([128, C], f32, name="py", tag="pt")
            for t in range(T2):
                lhsT = hT[:, t, :, mt * 128:(mt + 1) * 128]
                for hc in range(3):
                    # hc0+hc1 share PSUM bank 0, hc2 owns bank 1
                    nc.tensor.matmul(
                        py[:, hc * 256:(hc + 1) * 256],
                        lhsT=lhsT,
                        rhs=w2s[t // 2][:, t % 2, :,
                                        hc * 256:(hc + 1) * 256],
                        start=(t == 0 and hc != 1),
                        stop=(t == T2 - 1 and hc != 0),
                        perf_mode=DR,
                    )
            # residual add in place (DVE: the Pool engine cannot read PSUM)
            z = xns[mt]
            nc.vector.tensor_add(z, py[:, 0:H], z)
            stats = spool.tile([128, 3, 6], f32, name="stats", tag="stats")
            # g2 first: its half is ready as soon as the DVE add retires
            nc.vector.bn_stats(stats[:, 2], z[:, 512:768])
            nc.vector.bn_stats(stats[:, 0], z[:, 0:256])
            nc.vector.bn_stats(stats[:, 1], z[:, 256:512])
            mv = spool.tile([128, 2], f32, name="mv", tag="mv")
            nc.vector.bn_aggr(mv, stats)
            rstd = spool.tile([128, 1], f32, name="rstd", tag="rstd")
            nc.scalar.activation(rstd, mv[:, 1:2], AF.Sqrt, bias=epsT)
            nc.vector.reciprocal(out=rstd, in_=rstd)
            # normalize as z*rstd + (-mean*rstd) on the Scalar engine,
            # stores on the (idle) sync queue; halves overlap the tail
            nmr = spool.tile([128, 1], f32, name="nmr", tag="nmr")
            nc.vector.tensor_scalar(
                out=nmr, in0=mv[:, 0:1], scalar1=rstd, scalar2=-1.0,
                op0=ALU.mult, op1=ALU.mult)
            zo = zpool.tile([128, H], f32, name="zot", tag="zot")
            for h0 in (0, H // 2):
                sl = slice(h0, h0 + H // 2)
                nc.scalar.activation(zo[:, sl], z[:, sl], AF.Identity,
                                     bias=nmr, scale=rstd)
                nc.sync.dma_start(out=out[:, mt, sl], in_=zo[:, sl])

    nc.compile()
    return nc


def _get_nc(act_name="Gelu"):
    key = ("nc", act_name)
    if key not in _CACHE:
        _CACHE[key] = _build_nc(act_name)
    return _CACHE[key]


def _shard_inputs(x, task_ids, W1, b1, W2, b2):
    """Host-side dispatch: stable-sort tokens by expert id, chunk into E
    equal capacity-C blocks (exactly the reference's xs = x[order].reshape),
    then quantize/pack into the DoubleRow fp8 operand layouts."""
    import ml_dtypes

    f8 = ml_dtypes.float8_e4m3
    expert = (task_ids.astype(np.int64) % E).astype(np.int32)
    order = np.argsort(expert, kind="stable")
    xs = x[order]
    in_maps = []
    for e in range(E):
        xe = xs[e * C:(e + 1) * C]                       # [C, H]
        # xd[p, t, j, c] = fp8(xe[c, 256t + 128j + p])
        xq = np.ascontiguousarray(xe.T).astype(f8)
        xd = xq.reshape(T1, 2, 128, C).transpose(2, 0, 1, 3)
        # w1[p, it, t, j, m] = fp8(512 * W1[256t+128j+p, 128it+m])
        w1q = (W1[e] * WS).astype(f8)
        w1d = w1q.reshape(T1, 2, 128, IT, 128).transpose(2, 3, 0, 1, 4)
        # w2[p, tp, ts, j, h] = fp8(512 * W2[512tp+256ts+128j+p, h])
        w2q = (W2[e] * WS).astype(f8)
        w2d = w2q.reshape(T2 // 2, 2, 2, 128, H).transpose(3, 0, 1, 2, 4)
        b1t = b1[e].reshape(IT, 128).T
        xnv = ((xe + b2[e][None, :]) * WS).reshape(MT, 128, H).transpose(1, 0, 2)
        in_maps.append({
            "xd": np.ascontiguousarray(xd),
            "w1": np.ascontiguousarray(w1d),
            "w2": np.ascontiguousarray(w2d),
            "b1t": np.ascontiguousarray(b1t, dtype=np.float32),
            "xn": np.ascontiguousarray(xnv, dtype=np.float32),
        })
    return in_maps, order


def kernel(x, task_ids, W1, b1, W2, b2, gamma, beta):
    from concourse import bass_utils

    x = np.asarray(x, dtype=np.float32)
    task_ids = np.asarray(task_ids)
    W1 = np.asarray(W1, dtype=np.float32)
    b1 = np.asarray(b1, dtype=np.float32)
    W2 = np.asarray(W2, dtype=np.float32)
    b2 = np.asarray(b2, dtype=np.float32)
    gamma = np.asarray(gamma, dtype=np.float32)
    beta = np.asarray(beta, dtype=np.float32)

    in_maps, order = _shard_inputs(x, task_ids, W1, b1, W2, b2)
    nc = _get_nc()
    res = bass_utils.run_bass_kernel_spmd(nc, in_maps, core_ids=list(range(E)))
    _CACHE["last_results"] = res

    z = np.concatenate(
        [res.results[e]["out"].transpose(1, 0, 2).reshape(C, H) for e in range(E)],
        axis=0)
    # per-expert gamma/beta (identity for this problem's inputs; applied on
    # host only when nontrivial, matching the reference's z*gamma + beta)
    if not (np.all(gamma == 1.0) and np.all(beta == 0.0)):
        blk = np.repeat(np.arange(E), C)  # reference uses capacity blocks
        z = z * gamma[blk] + beta[blk]
    out = np.empty((N, H), dtype=np.float32)
    out[order] = z
    return out


# revision 15
# speedup vs baseline: 1.1836x; 1.0941x over previous
"""MoE FFN (BertGeneration-style) on 8 TRN2 NeuronCores, expert-parallel.

Problem: 8192 tokens, expert = task_id % 8, per-expert FFN
(768 -> 3072 gelu -> 768) + residual + per-expert LayerNorm.

Strategy: routing (dispatch/combine) is a host-side permutation; each of the
8 cores runs one expert's FFN over its 1024-token block.  Both GEMMs run in
fp8 (e4m3) with MatmulPerfMode.DoubleRow: a [128, 2, 128] stationary tile
contracts K=256 per instruction at 2x the bf16/fp32r MAC rate, and fp8
stationary tiles load 4x faster than fp32r ones (the fp32r baseline was
LdWeights-bound).

Accuracy: weights are pre-scaled by 512 so uniform(+-1/sqrt(H)) values sit in
e4m3's normal range (unscaled, most of W2 would land in subnormals).  The W1
scale is undone by the gelu activation's input scale (1/512); the W2 scale is
ironed out by pre-scaling the residual by 512 -- LayerNorm is scale-invariant
so the output is unchanged.  Measured rel err ~1e-2 vs the fp32 reference.

Schedule notes (from HW traces):
- phase 1 is a single dense matmul stream; its w1 chunk stream is the
  critical DMA path, so w2/xn prefetches are pinned behind phase progress
  (dummy-copy deps) to keep them from oversubscribing HBM mid-phase --
  a starved w1 stream stalls the PE and drops its pstate clock.
- the LayerNorm epilogue is split across engines (residual add: Pool+DVE,
  stats: DVE, sqrt + normalize: Scalar, stores: Sync queue) so the trailing
  chain after the last matmul is short and no single engine backs up.
"""

import sys

if "/opt/trn_rl_repo" not in sys.path:
    sys.path.insert(0, "/opt/trn_rl_repo")

import numpy as np


def _install_axon_hooks_shim():
    """Provide antenv.axon_hooks (NTFF profiling hook) when the image's
    antenv lacks it — a thin ctypes wrapper over libaxon_pjrt.so, matching
    trn_agent_boot.trn_boot._ntff_profile_via_ctypes.  Only exercised when
    profiling is requested (BASS_TRACE); harmless otherwise."""
    import contextlib
    import ctypes
    import types

    try:
        import antenv.axon_hooks  # noqa: F401
        return
    except ImportError:
        pass
    try:
        import antenv
    except ImportError:
        return

    mod = types.ModuleType("antenv.axon_hooks")
    _state = {"hook": None, "init": False}

    def set_axon_ntff_profile_hook(h):
        _state["hook"] = h
        _state["init"] = True

    def get_axon_ntff_profile_hook():
        if _state["init"]:
            return _state["hook"]
        _state["init"] = True
        try:
            lib = ctypes.CDLL("/opt/axon/libaxon_pjrt.so")
        except OSError:
            return None
        if not hasattr(lib, "axon_start_nrt_profile"):
            return None
        lib.axon_start_nrt_profile.argtypes = [
            ctypes.POINTER(ctypes.c_int64), ctypes.c_size_t]
        lib.axon_start_nrt_profile.restype = ctypes.c_int64
        lib.axon_stop_nrt_profile.argtypes = [ctypes.c_char_p]
        lib.axon_stop_nrt_profile.restype = ctypes.c_int64

        @contextlib.contextmanager
        def _hook(output_dir, device_ids):
            import jax
            jax.devices()
            if device_ids:
                ids = (ctypes.c_int64 * len(device_ids))(*device_ids)
                rc = lib.axon_start_nrt_profile(ids, len(device_ids))
            else:
                rc = lib.axon_start_nrt_profile(None, 0)
            if rc != 0:
                raise RuntimeError(f"axon_start_nrt_profile rc={rc}")
            try:
                yield
            finally:
                n = lib.axon_stop_nrt_profile(str(output_dir).encode())
                print(f"profile: {n} file(s) written to {output_dir}")

        _state["hook"] = _hook
        return _hook

    mod.set_axon_ntff_profile_hook = set_axon_ntff_profile_hook
    mod.get_axon_ntff_profile_hook = get_axon_ntff_profile_hook
    sys.modules["antenv.axon_hooks"] = mod
    antenv.axon_hooks = mod


_install_axon_hooks_shim()

E = 8
N = 8192
H = 768
I = 3072
C = N // E        # 1024 tokens per expert/core
MT = C // 128     # 8   m-tiles (token dim per core)
IT = I // 128     # 24  i-tiles (intermediate dim)
T1 = H // 256     # 3   K-tiles of 256 for GEMM1 (hidden dim)
T2 = I // 256     # 12  K-tiles of 256 for GEMM2 (intermediate dim)
EPS = 1e-12
WS = 512.0        # weight pre-scale: keeps uniform(+-1/sqrt(H)) out of
                  # e4m3 subnormals; undone via gelu input scale / LN

_CACHE = {}


def _build_nc(act_name="Gelu"):
    from contextlib import ExitStack

    import concourse.tile as tile
    from concourse import bacc, mybir

    f32 = mybir.dt.float32
    f8 = mybir.dt.float8e4
    AF = mybir.ActivationFunctionType
    act_fn = getattr(AF, act_name)
    ALU = mybir.AluOpType
    DR = mybir.MatmulPerfMode.DoubleRow

    nc = bacc.Bacc("TRN2", target_bir_lowering=False, debug=False, num_devices=8)

    # DoubleRow operand layouts pack the K=256 contraction as (pair j, 128
    # partitions p): element [p, ..., j, ...] holds row k = 256*t + 128*j + p.
    xd = nc.dram_tensor("xd", [128, T1, 2, C], f8, kind="ExternalInput").ap()
    w1 = nc.dram_tensor("w1", [128, IT, T1, 2, 128], f8,
                        kind="ExternalInput").ap()
    w2 = nc.dram_tensor("w2", [128, T2 // 2, 2, 2, H], f8,
                        kind="ExternalInput").ap()
    b1t = nc.dram_tensor("b1t", [128, IT], f32, kind="ExternalInput").ap()
    xn = nc.dram_tensor("xn", [128, MT, H], f32, kind="ExternalInput").ap()
    out = nc.dram_tensor("out", [128, MT, H], f32, kind="ExternalOutput").ap()

    with ExitStack() as ctx:
        tc = ctx.enter_context(tile.TileContext(nc))
        persist = ctx.enter_context(tc.tile_pool(name="persist", bufs=1))
        psum = ctx.enter_context(tc.tile_pool(name="psum", bufs=4, space="PSUM"))
        w1pool = ctx.enter_context(tc.tile_pool(name="w1s", bufs=16))
        zpool = ctx.enter_context(tc.tile_pool(name="zo", bufs=3))
        spool = ctx.enter_context(tc.tile_pool(name="small", bufs=8))

        # hT doubles as GEMM1 output and GEMM2 stationary: [p, t, j, c] is
        # h[i=256t+128j+p, c], exactly DoubleRow's paired-K layout
        hTab = [persist.tile([128, T2 // 2, 2, C], f8, name=f"hT{i}")
                for i in range(2)]
        # x chunk for K-tile 0 lands as two half-token tiles so the first
        # matmuls fire ~0.4us earlier
        xk0h = [persist.tile([128, 2, 512], f8, name=f"xk0{h}", tag=f"xk0{h}")
                for h in range(2)]
        xk = [None] + [persist.tile([128, 2, C], f8, name=f"xk{t}", tag=f"xk{t}")
                       for t in (1, 2)]
        w2s = [persist.tile([128, 2, 2, H], f8, name=f"w2c{t}", tag=f"w2c{t}")
               for t in range(T2 // 2)]
        xns = [persist.tile([128, H], f32, name=f"xn{m}", tag=f"xn{m}")
               for m in range(MT)]
        b1s = persist.tile([128, IT], f32, name="b1s")
        epsT = persist.tile([128, 1], f32, name="epsT")

        nc.vector.memset(epsT, EPS)

        # ---- phase 1: hT = gelu((W1s.T @ xT)/512 + b1) ----
        # startup-critical pair: xk0h[0] leads the gpsimd queue, w1[0] leads
        # the sync queue; nothing else competes for HBM until they land.
        # b1 rides the otherwise-idle scalar queue.
        nc.gpsimd.dma_start(out=xk0h[0], in_=xd[:, 0, :, 0:512])
        nc.gpsimd.dma_start(out=xk0h[1], in_=xd[:, 0, :, 512:C])
        nc.gpsimd.dma_start(out=xk[1], in_=xd[:, 1])
        nc.gpsimd.dma_start(out=xk[2], in_=xd[:, 2])
        nc.scalar.dma_start(out=b1s, in_=b1t)

        for it in range(IT):
            w1t = w1pool.tile([128, T1, 2, 128], f8, name="w1t", tag="w1t")
            nc.sync.dma_start(out=w1t, in_=w1[:, it])
            ph = psum.tile([128, C], f32, name="ph", tag="pt")
            for t in range(T1):
                lhsT = w1t[:, t, :, :]
                for cc in range(4):
                    if t == 0:
                        rhs = xk0h[cc // 2][:, :, (cc % 2) * 256:
                                            (cc % 2) * 256 + 256]
                    else:
                        rhs = xk[t][:, :, cc * 256:(cc + 1) * 256]
                    # PSUM groups are bank-granular (2KB): within a bank
                    # (2 x 256-token chunks) only the first chunk starts the
                    # group and only the last stops it; the middle chunk's
                    # first write lands on pending-zero bytes
                    nc.tensor.matmul(
                        ph[:, cc * 256:(cc + 1) * 256],
                        lhsT=lhsT,
                        rhs=rhs,
                        start=(t == 0 and cc % 2 == 0),
                        stop=(t == T1 - 1 and cc % 2 == 1),
                        perf_mode=DR,
                    )
            tt = it // 2
            hdst = hTab[tt // (T2 // 2)][:, tt % (T2 // 2), it % 2, :]
            nc.scalar.activation(hdst, ph, act_fn,
                                 bias=b1s[:, it:it + 1], scale=1.0 / WS)
            # W2 chunks + first two residual blocks stream on the gpsimd
            # queue as a completion-paced chain: each link's DMA is pinned
            # (dummy copy -> WAW) behind the PREVIOUS link's arrival, first
            # link behind xk[2].  The hardware DMA rings are FIFO in issue
            # order, so an unpaced prefetch burst would head-of-line block
            # the critical w1 stream; pacing keeps at most one 393KB chunk
            # ahead of it.  Pin sources are tiles nothing writes later
            # (reading hT would hand every subsequent gelu a WAR dep,
            # backing up the PSUM ring).
            if it == 0:
                chain = [("w2", i) for i in range(3)] + [("xn", 0)] + \
                        [("w2", i) for i in (3, 4)] + [("xn", 1), ("w2", 5)]
                prev = xk[2][:, 0, 0:4].bitcast(f32)
                for kind, i in chain:
                    if kind == "w2":
                        dst, src = w2s[i], w2[:, i]
                        pin = w2s[i][:, 0, 0, 0:4].bitcast(f32)
                    else:
                        dst, src = xns[i], xn[:, i]
                        pin = xns[i][:, 0:1]
                    nc.vector.tensor_copy(out=pin, in_=prev)
                    nc.gpsimd.dma_start(out=dst, in_=src)
                    prev = pin

        # ---- phase 2: y = hT.T @ W2s; z = y + 512*(x + b2); LayerNorm ----
        for mt in range(MT):
            # prefetch the residual block two iterations ahead, pinned
            # behind this iteration's (already-arrived) block
            if mt + 2 < MT:
                nc.vector.tensor_copy(out=xns[mt + 2][:, 0:1],
                                      in_=xns[mt][:, 0:1])
                nc.gpsimd.dma_start(out=xns[mt + 2], in_=xn[:, mt + 2])
            py = psum.tile([128, C], f32, name="py", tag="pt")
            for t in range(T2):
                lhsT = hTab[t // (T2 // 2)][:, t % (T2 // 2), :,
                            mt * 128:(mt + 1) * 128]
                for hc in range(3):
                    # hc0+hc1 share PSUM bank 0, hc2 owns bank 1
                    nc.tensor.matmul(
                        py[:, hc * 256:(hc + 1) * 256],
                        lhsT=lhsT,
                        rhs=w2s[t // 2][:, t % 2, :,
                                        hc * 256:(hc + 1) * 256],
                        start=(t == 0 and hc != 1),
                        stop=(t == T2 - 1 and hc != 0),
                        perf_mode=DR,
                    )
            # residual add in place (DVE: the Pool engine cannot read PSUM)
            z = xns[mt]
            nc.vector.tensor_add(z, py[:, 0:H], z)
            stats = spool.tile([128, 3, 6], f32, name="stats", tag="stats")
            # g2 first: its half is ready as soon as the DVE add retires
            nc.vector.bn_stats(stats[:, 2], z[:, 512:768])
            nc.vector.bn_stats(stats[:, 0], z[:, 0:256])
            nc.vector.bn_stats(stats[:, 1], z[:, 256:512])
            mv = spool.tile([128, 2], f32, name="mv", tag="mv")
            nc.vector.bn_aggr(mv, stats)
            rstd = spool.tile([128, 1], f32, name="rstd", tag="rstd")
            nc.scalar.activation(rstd, mv[:, 1:2], AF.Sqrt, bias=epsT)
            nc.vector.reciprocal(out=rstd, in_=rstd)
            # normalize as z*rstd + (-mean*rstd) on the Scalar engine,
            # stores on the (idle) sync queue; halves overlap the tail
            nmr = spool.tile([128, 1], f32, name="nmr", tag="nmr")
            nc.vector.tensor_scalar(
                out=nmr, in0=mv[:, 0:1], scalar1=rstd, scalar2=-1.0,
                op0=ALU.mult, op1=ALU.mult)
            zo = zpool.tile([128, H], f32, name="zot", tag="zot")
            for h0 in (0, H // 2):
                sl = slice(h0, h0 + H // 2)
                nc.scalar.activation(zo[:, sl], z[:, sl], AF.Identity,
                                     bias=nmr, scale=rstd)
                nc.sync.dma_start(out=out[:, mt, sl], in_=zo[:, sl])

    nc.compile()
    return nc


def _get_nc(act_name="Gelu"):
    key = ("nc", act_name)
    if key not in _CACHE:
        _CACHE[key] = _build_nc(act_name)
    return _CACHE[key]


def _shard_inputs(x, task_ids, W1, b1, W2, b2):
    """Host-side dispatch: stable-sort tokens by expert id, chunk into E
    equal capacity-C blocks (exactly the reference's xs = x[order].reshape),
    then quantize/pack into the DoubleRow fp8 operand layouts."""
    import ml_dtypes

    f8 = ml_dtypes.float8_e4m3
    expert = (task_ids.astype(np.int64) % E).astype(np.int32)
    order = np.argsort(expert, kind="stable")
    xs = x[order]
    in_maps = []
    for e in range(E):
        xe = xs[e * C:(e + 1) * C]                       # [C, H]
        # xd[p, t, j, c] = fp8(xe[c, 256t + 128j + p])
        xq = np.ascontiguousarray(xe.T).astype(f8)
        xd = xq.reshape(T1, 2, 128, C).transpose(2, 0, 1, 3)
        # w1[p, it, t, j, m] = fp8(512 * W1[256t+128j+p, 128it+m])
        w1q = (W1[e] * WS).astype(f8)
        w1d = w1q.reshape(T1, 2, 128, IT, 128).transpose(2, 3, 0, 1, 4)
        # w2[p, tp, ts, j, h] = fp8(512 * W2[512tp+256ts+128j+p, h])
        w2q = (W2[e] * WS).astype(f8)
        w2d = w2q.reshape(T2 // 2, 2, 2, 128, H).transpose(3, 0, 1, 2, 4)
        b1t = b1[e].reshape(IT, 128).T
        xnv = ((xe + b2[e][None, :]) * WS).reshape(MT, 128, H).transpose(1, 0, 2)
        in_maps.append({
            "xd": np.ascontiguousarray(xd),
            "w1": np.ascontiguousarray(w1d),
            "w2": np.ascontiguousarray(w2d),
            "b1t": np.ascontiguousarray(b1t, dtype=np.float32),
            "xn": np.ascontiguousarray(xnv, dtype=np.float32),
        })
    return in_maps, order


def kernel(x, task_ids, W1, b1, W2, b2, gamma, beta):
    from concourse import bass_utils

    x = np.asarray(x, dtype=np.float32)
    task_ids = np.asarray(task_ids)
    W1 = np.asarray(W1, dtype=np.float32)
    b1 = np.asarray(b1, dtype=np.float32)
    W2 = np.asarray(W2, dtype=np.float32)
    b2 = np.asarray(b2, dtype=np.float32)
    gamma = np.asarray(gamma, dtype=np.float32)
    beta = np.asarray(beta, dtype=np.float32)

    in_maps, order = _shard_inputs(x, task_ids, W1, b1, W2, b2)
    nc = _get_nc()
    res = bass_utils.run_bass_kernel_spmd(nc, in_maps, core_ids=list(range(E)))
    _CACHE["last_results"] = res

    z = np.concatenate(
        [res.results[e]["out"].transpose(1, 0, 2).reshape(C, H) for e in range(E)],
        axis=0)
    # per-expert gamma/beta (identity for this problem's inputs; applied on
    # host only when nontrivial, matching the reference's z*gamma + beta)
    if not (np.all(gamma == 1.0) and np.all(beta == 0.0)):
        blk = np.repeat(np.arange(E), C)  # reference uses capacity blocks
        z = z * gamma[blk] + beta[blk]
    out = np.empty((N, H), dtype=np.float32)
    out[order] = z
    return out
